# revision 1
# baseline (speedup 1.0000x reference)
"""Trainium2 Bass kernel for nn_PoolerYOLO (multi-level ROIAlign pooling).

Strategy
--------
Host side (index/descriptor prep, negligible FLOPs):
  * Build a patch table: for every (batch, level, y, x) the 2x2 pixel
    neighborhood [(y,x), (y,x+1c), (y+1c,x), (y+1c,x+1c)] (c = clamped) is
    stored contiguously as 4*C floats.  One gather therefore fetches all 4
    bilinear corners of one sample point in a single 4 KiB contiguous block.
  * For each ROI compute (in float32, mirroring the reference op-for-op)
    the 4 sample-point patch indices and 16 bilinear weights per output bin.
    Bins whose 16 weights are all zero (out-of-bounds ROIs — the majority)
    are skipped entirely; their output is exactly 0, as in the reference.
  * Active bins are round-robined across the 8 NeuronCores (data parallel
    over bins; the feature patch table is replicated per core).

Device side (all heavy data movement + arithmetic):
  Per chunk of 128 bins: 4 indirect DMA gathers (one per sample point,
  128 x 4KiB blocks), a broadcast multiply by the 16 weights, a 16-way
  strided add-reduce to [128 bins, 256 ch], and a contiguous store.

Host side (unshard): scatter the dense per-core [bins, 256] results into
the full [N, 256, 7, 7] output (zeros elsewhere).
"""
import numpy as np

OUT = 7
SR = 2
IMG = 640
SCALES = (80, 40, 20)
NCORES = 8
P = 128
C = 256
NSAMP = 4          # sample points per bin (SR*SR)
BLK = 4            # pixels per patch block
K = NSAMP * BLK    # weighted contributions per bin


def _build_patch_table(feats):
    """[sum_l B*H_l*W_l, BLK*C] f32 patch table + per-level row bases."""
    tables, bases = [], []
    base = 0
    for lvl, feat in enumerate(feats):
        f = np.asarray(feat, dtype=np.float32)
        B, Cc, H, W = f.shape
        nhwc = np.ascontiguousarray(f.transpose(0, 2, 3, 1))       # [B,H,W,C]
        yp = np.minimum(np.arange(H) + 1, H - 1)
        xp = np.minimum(np.arange(W) + 1, W - 1)
        p00 = nhwc
        p01 = nhwc[:, :, xp, :]
        p10 = nhwc[:, yp, :, :]
        p11 = nhwc[:, yp][:, :, xp]
        patch = np.stack([p00, p01, p10, p11], axis=3)             # [B,H,W,4,C]
        tables.append(patch.reshape(B * H * W, BLK * Cc))
        bases.append(base)
        base += B * H * W
    return np.ascontiguousarray(np.concatenate(tables, axis=0)), bases


def _prep_indices_weights(boxes, batch_ids, feat_idx):
    """Per-bin patch indices [N,7,7,4] i32 and weights [N,7,7,16] f32.

    Float32 arithmetic mirrors reference.py exactly (same ops, same order)
    so the validity comparisons are bit-identical.
    """
    bx = np.asarray(boxes, dtype=np.float32)
    bids = np.asarray(batch_ids).astype(np.int64)
    fidx = np.asarray(feat_idx).astype(np.int64)
    N = bx.shape[0]
    t0 = SCALES[0] ** 2 - 1
    t1 = SCALES[0] ** 2 + SCALES[1] ** 2 - 1
    levels = np.where(fidx < t0, 2, np.where(fidx < t1, 1, 0))

    off = (np.arange(OUT, dtype=np.float32)[:, None]
           + (np.arange(SR, dtype=np.float32) + np.float32(0.5))[None, :]
           / np.float32(SR)).reshape(-1)                            # [14]

    idx_all = np.zeros((N, OUT, OUT, NSAMP), np.int32)
    w_all = np.zeros((N, OUT, OUT, K), np.float32)
    # bases must match _build_patch_table's level order
    hws = [2 * s * s for s in SCALES]
    bases = [0, hws[0], hws[0] + hws[1]]

    for lvl in range(3):
        m = levels == lvl
        if not m.any():
            continue
        H = W = SCALES[lvl]
        s = np.float32(IMG / SCALES[lvl])
        b = bx[m]
        x1 = b[:, 0] * s
        y1 = b[:, 1] * s
        x2 = b[:, 2] * s
        y2 = b[:, 3] * s
        roi_w = np.maximum(x2 - x1, np.float32(1.0))
        roi_h = np.maximum(y2 - y1, np.float32(1.0))
        bin_w = roi_w / np.float32(OUT)
        bin_h = roi_h / np.float32(OUT)
        ys = y1[:, None] + bin_h[:, None] * off[None, :]            # [n,14]
        xs = x1[:, None] + bin_w[:, None] * off[None, :]
        vy = (ys > -1.0) & (ys < H)
        vx = (xs > -1.0) & (xs < W)
        yc = np.clip(ys, np.float32(0.0), np.float32(H - 1))
        xc = np.clip(xs, np.float32(0.0), np.float32(W - 1))
        y0 = np.floor(yc).astype(np.int32)
        x0 = np.floor(xc).astype(np.int32)
        ly = yc - y0.astype(np.float32)
        lx = xc - x0.astype(np.float32)
        hy = np.float32(1.0) - ly
        hx = np.float32(1.0) - lx

        bl = bids[m]
        pidx = (bases[lvl] + (bl[:, None, None] * H + y0[:, :, None]) * W
                + x0[:, None, :]).astype(np.int32)                  # [n,14,14]
        valid = (vy[:, :, None] & vx[:, None, :])                   # [n,14,14]
        wy = np.stack([hy, ly], -1)                                 # [n,14,2]
        wx = np.stack([hx, lx], -1)                                 # [n,14,2]
        w4 = (wy[:, :, None, :, None] * wx[:, None, :, None, :])    # [n,py,px,cy,cx]
        w4 = np.where(valid[:, :, :, None, None], w4, np.float32(0.0))
        w4 = w4 * np.float32(0.25)

        n = b.shape[0]
        pidx_b = pidx.reshape(n, OUT, SR, OUT, SR).transpose(0, 1, 3, 2, 4)
        idx_all[m] = pidx_b.reshape(n, OUT, OUT, NSAMP)
        w_b = w4.reshape(n, OUT, SR, OUT, SR, 2, 2).transpose(0, 1, 3, 2, 4, 5, 6)
        w_all[m] = w_b.reshape(n, OUT, OUT, K)

    return idx_all, w_all


_NEFF_CACHE = {}


def _build_program(nrows, nch):
    """Bass program: nch chunks of 128 bins; identical for every core."""
    import concourse.bass as bass
    import concourse.bacc as bacc
    import concourse.tile as tile
    from concourse import mybir

    nc = bacc.Bacc("TRN2", target_bir_lowering=False, debug=False,
                   enable_asserts=False, num_devices=NCORES)
    table_d = nc.dram_tensor("table", [nrows, BLK * C], mybir.dt.float32,
                             kind="ExternalInput")
    idx_d = nc.dram_tensor("idx", [nch, P, NSAMP], mybir.dt.int32,
                           kind="ExternalInput")
    w_d = nc.dram_tensor("w", [nch, P, K], mybir.dt.float32,
                         kind="ExternalInput")
    out_d = nc.dram_tensor("out", [nch * P, C], mybir.dt.float32,
                           kind="ExternalOutput")

    with tile.TileContext(nc) as tc:
        with tc.tile_pool(name="io", bufs=4) as io_pool, \
             tc.tile_pool(name="g", bufs=3) as g_pool, \
             tc.tile_pool(name="t", bufs=2) as t_pool, \
             tc.tile_pool(name="r", bufs=2) as r_pool:
            for ch in range(nch):
                idx_t = io_pool.tile([P, NSAMP], mybir.dt.int32)
                nc.sync.dma_start(out=idx_t[:], in_=idx_d.ap()[ch])
                w_t = io_pool.tile([P, K], mybir.dt.float32)
                nc.sync.dma_start(out=w_t[:], in_=w_d.ap()[ch])

                g_t = g_pool.tile([P, K * C], mybir.dt.float32)
                for s in range(NSAMP):
                    nc.gpsimd.indirect_dma_start(
                        out=g_t[:, s * BLK * C:(s + 1) * BLK * C],
                        out_offset=None,
                        in_=table_d.ap()[:],
                        in_offset=bass.IndirectOffsetOnAxis(
                            ap=idx_t[:, s:s + 1], axis=0),
                    )

                g_v = g_t[:].rearrange("p (k c) -> p k c", c=C)
                t_t = t_pool.tile([P, K, C], mybir.dt.float32)
                w_b = w_t[:].unsqueeze(2).to_broadcast([P, K, C])
                nc.vector.tensor_tensor(out=t_t[:], in0=g_v, in1=w_b,
                                        op=mybir.AluOpType.mult)

                r_t = r_pool.tile([P, C], mybir.dt.float32)
                nc.vector.tensor_reduce(out=r_t[:],
                                        in_=t_t[:].transpose([0, 2, 1]),
                                        axis=mybir.AxisListType.X,
                                        op=mybir.AluOpType.add)

                nc.sync.dma_start(out=out_d.ap()[ch * P:(ch + 1) * P, :],
                                  in_=r_t[:])
    nc.compile()
    return nc


def _run(inputs, trace=False, trace_cores=None):
    """Returns (full_output [N,C,7,7] f32, exec_time_ns or None)."""
    from concourse.bass_utils import run_bass_kernel_spmd

    feats = (inputs["feat0"], inputs["feat1"], inputs["feat2"])
    boxes = inputs["boxes"]
    batch_ids = inputs["batch_ids"]
    feat_idx = inputs["feat_idx"]
    N = np.asarray(boxes).shape[0]

    table, _bases = _build_patch_table(feats)
    idx_all, w_all = _prep_indices_weights(boxes, batch_ids, feat_idx)

    act = (w_all != 0).any(-1)                      # [N,7,7]
    rois, phs, pws = np.nonzero(act)
    nact = len(rois)
    out_full = np.zeros((N, C, OUT, OUT), np.float32)
    if nact == 0:
        return out_full, None

    order = np.arange(nact)
    core_of = order % NCORES
    per_core = [np.nonzero(core_of == c)[0] for c in range(NCORES)]
    maxn = max(len(p) for p in per_core)
    nch = (maxn + P - 1) // P

    key = (table.shape[0], nch)
    if key not in _NEFF_CACHE:
        _NEFF_CACHE[key] = _build_program(table.shape[0], nch)
    nc = _NEFF_CACHE[key]

    in_maps = []
    for c in range(NCORES):
        sel = per_core[c]
        icore = np.zeros((nch * P, NSAMP), np.int32)
        wcore = np.zeros((nch * P, K), np.float32)
        icore[:len(sel)] = idx_all[rois[sel], phs[sel], pws[sel]]
        wcore[:len(sel)] = w_all[rois[sel], phs[sel], pws[sel]]
        in_maps.append({
            "table": table,
            "idx": icore.reshape(nch, P, NSAMP),
            "w": wcore.reshape(nch, P, K),
        })

    kwargs = {}
    if trace:
        kwargs["trace"] = True
        if trace_cores is not None:
            kwargs["trace_cores"] = trace_cores
    res = run_bass_kernel_spmd(nc, in_maps, core_ids=list(range(NCORES)),
                               **kwargs)

    for c in range(NCORES):
        sel = per_core[c]
        r = res.results[c]["out"][:len(sel)]
        out_full[rois[sel], :, phs[sel], pws[sel]] = r

    return out_full, getattr(res, "exec_time_ns", None)


def kernel(**inputs):
    out, _ = _run(inputs, trace=False)
    return out


# revision 3
# speedup vs baseline: 1.3518x; 1.3518x over previous
"""Trainium2 Bass kernel for nn_PoolerYOLO (multi-level ROIAlign pooling).

Strategy
--------
Host side (index/descriptor prep, negligible FLOPs):
  * Build a patch table: for every (batch, level, y, x) the 2x2 pixel
    neighborhood [(y,x), (y,x+1c), (y+1c,x), (y+1c,x+1c)] (c = clamped) is
    stored contiguously as 4*C floats.  One gather therefore fetches all 4
    bilinear corners of one sample point in a single 4 KiB contiguous block.
  * For each ROI compute (in float32, mirroring the reference op-for-op)
    the 4 sample-point patch indices and 16 bilinear weights per output bin.
    Bins whose 16 weights are all zero (out-of-bounds ROIs — the majority)
    are skipped entirely; their output is exactly 0, as in the reference.
  * Active bins are round-robined across the 8 NeuronCores (data parallel
    over bins; the feature patch table is replicated per core).

Device side (all heavy data movement + arithmetic):
  Per chunk of 128 bins: 4 indirect DMA gathers (one per sample point,
  128 x 4KiB blocks), a broadcast multiply by the 16 weights, a 16-way
  strided add-reduce to [128 bins, 256 ch], and a contiguous store.

Host side (unshard): scatter the dense per-core [bins, 256] results into
the full [N, 256, 7, 7] output (zeros elsewhere).
"""
import numpy as np

OUT = 7
SR = 2
IMG = 640
SCALES = (80, 40, 20)
NCORES = 8
P = 128
C = 256
NSAMP = 4          # sample points per bin (SR*SR)
BLK = 4            # pixels per patch block
K = NSAMP * BLK    # weighted contributions per bin


def _build_patch_table(feats):
    """[sum_l B*H_l*W_l, BLK*C] f32 patch table + per-level row bases."""
    tables, bases = [], []
    base = 0
    for lvl, feat in enumerate(feats):
        f = np.asarray(feat, dtype=np.float32)
        B, Cc, H, W = f.shape
        nhwc = np.ascontiguousarray(f.transpose(0, 2, 3, 1))       # [B,H,W,C]
        yp = np.minimum(np.arange(H) + 1, H - 1)
        xp = np.minimum(np.arange(W) + 1, W - 1)
        p00 = nhwc
        p01 = nhwc[:, :, xp, :]
        p10 = nhwc[:, yp, :, :]
        p11 = nhwc[:, yp][:, :, xp]
        patch = np.stack([p00, p01, p10, p11], axis=3)             # [B,H,W,4,C]
        tables.append(patch.reshape(B * H * W, BLK * Cc))
        bases.append(base)
        base += B * H * W
    return np.ascontiguousarray(np.concatenate(tables, axis=0)), bases


def _prep_indices_weights(boxes, batch_ids, feat_idx):
    """Per-bin patch indices [N,7,7,4] i32 and weights [N,7,7,16] f32.

    Float32 arithmetic mirrors reference.py exactly (same ops, same order)
    so the validity comparisons are bit-identical.
    """
    bx = np.asarray(boxes, dtype=np.float32)
    bids = np.asarray(batch_ids).astype(np.int64)
    fidx = np.asarray(feat_idx).astype(np.int64)
    N = bx.shape[0]
    t0 = SCALES[0] ** 2 - 1
    t1 = SCALES[0] ** 2 + SCALES[1] ** 2 - 1
    levels = np.where(fidx < t0, 2, np.where(fidx < t1, 1, 0))

    off = (np.arange(OUT, dtype=np.float32)[:, None]
           + (np.arange(SR, dtype=np.float32) + np.float32(0.5))[None, :]
           / np.float32(SR)).reshape(-1)                            # [14]

    idx_all = np.zeros((N, OUT, OUT, NSAMP), np.int32)
    w_all = np.zeros((N, OUT, OUT, K), np.float32)
    # bases must match _build_patch_table's level order
    hws = [2 * s * s for s in SCALES]
    bases = [0, hws[0], hws[0] + hws[1]]

    for lvl in range(3):
        m = levels == lvl
        if not m.any():
            continue
        H = W = SCALES[lvl]
        s = np.float32(IMG / SCALES[lvl])
        b = bx[m]
        x1 = b[:, 0] * s
        y1 = b[:, 1] * s
        x2 = b[:, 2] * s
        y2 = b[:, 3] * s
        roi_w = np.maximum(x2 - x1, np.float32(1.0))
        roi_h = np.maximum(y2 - y1, np.float32(1.0))
        bin_w = roi_w / np.float32(OUT)
        bin_h = roi_h / np.float32(OUT)
        ys = y1[:, None] + bin_h[:, None] * off[None, :]            # [n,14]
        xs = x1[:, None] + bin_w[:, None] * off[None, :]
        vy = (ys > -1.0) & (ys < H)
        vx = (xs > -1.0) & (xs < W)
        yc = np.clip(ys, np.float32(0.0), np.float32(H - 1))
        xc = np.clip(xs, np.float32(0.0), np.float32(W - 1))
        y0 = np.floor(yc).astype(np.int32)
        x0 = np.floor(xc).astype(np.int32)
        ly = yc - y0.astype(np.float32)
        lx = xc - x0.astype(np.float32)
        hy = np.float32(1.0) - ly
        hx = np.float32(1.0) - lx

        bl = bids[m]
        pidx = (bases[lvl] + (bl[:, None, None] * H + y0[:, :, None]) * W
                + x0[:, None, :]).astype(np.int32)                  # [n,14,14]
        valid = (vy[:, :, None] & vx[:, None, :])                   # [n,14,14]
        wy = np.stack([hy, ly], -1)                                 # [n,14,2]
        wx = np.stack([hx, lx], -1)                                 # [n,14,2]
        w4 = (wy[:, :, None, :, None] * wx[:, None, :, None, :])    # [n,py,px,cy,cx]
        w4 = np.where(valid[:, :, :, None, None], w4, np.float32(0.0))
        w4 = w4 * np.float32(0.25)

        n = b.shape[0]
        pidx_b = pidx.reshape(n, OUT, SR, OUT, SR).transpose(0, 1, 3, 2, 4)
        idx_all[m] = pidx_b.reshape(n, OUT, OUT, NSAMP)
        w_b = w4.reshape(n, OUT, SR, OUT, SR, 2, 2).transpose(0, 1, 3, 2, 4, 5, 6)
        w_all[m] = w_b.reshape(n, OUT, OUT, K)

    return idx_all, w_all


_NEFF_CACHE = {}


def _build_program(nrows, nch):
    """Bass program: nch chunks of 128 bins; identical for every core."""
    import concourse.bass as bass
    import concourse.bacc as bacc
    import concourse.tile as tile
    from concourse import mybir

    nc = bacc.Bacc("TRN2", target_bir_lowering=False, debug=False,
                   enable_asserts=False, num_devices=NCORES)
    table_d = nc.dram_tensor("table", [nrows, BLK * C], mybir.dt.float32,
                             kind="ExternalInput")
    idx_d = nc.dram_tensor("idx", [nch, P, NSAMP], mybir.dt.int32,
                           kind="ExternalInput")
    w_d = nc.dram_tensor("w", [nch, P, K], mybir.dt.float32,
                         kind="ExternalInput")
    out_d = nc.dram_tensor("out", [nch * P, C], mybir.dt.float32,
                           kind="ExternalOutput")

    ACT_MULS = 8        # how many of the 16 weighted muls run on ScalarE

    with tile.TileContext(nc) as tc:
        with tc.tile_pool(name="io", bufs=4) as io_pool, \
             tc.tile_pool(name="g", bufs=3) as g_pool, \
             tc.tile_pool(name="t", bufs=2) as t_pool, \
             tc.tile_pool(name="r", bufs=2) as r_pool:
            for ch in range(nch):
                idx_t = io_pool.tile([P, NSAMP], mybir.dt.int32)
                nc.sync.dma_start(out=idx_t[:], in_=idx_d.ap()[ch])
                w_t = io_pool.tile([P, K], mybir.dt.float32)
                nc.sync.dma_start(out=w_t[:], in_=w_d.ap()[ch])

                g_t = g_pool.tile([P, K * C], mybir.dt.float32)
                for s in range(NSAMP):
                    nc.gpsimd.indirect_dma_start(
                        out=g_t[:, s * BLK * C:(s + 1) * BLK * C],
                        out_offset=None,
                        in_=table_d.ap()[:],
                        in_offset=bass.IndirectOffsetOnAxis(
                            ap=idx_t[:, s:s + 1], axis=0),
                    )

                # t[k] = g[k] * w[k]  (per-partition scalar weight per slice)
                t_t = t_pool.tile([P, K * C], mybir.dt.float32)
                for k in range(K):
                    sl = slice(k * C, (k + 1) * C)
                    if k < ACT_MULS:
                        nc.scalar.activation(
                            out=t_t[:, sl], in_=g_t[:, sl],
                            func=mybir.ActivationFunctionType.Copy,
                            scale=w_t[:, k:k + 1])
                    else:
                        nc.vector.tensor_scalar_mul(
                            t_t[:, sl], g_t[:, sl], w_t[:, k:k + 1])

                # contiguous in-place tree reduction over the 16 slices
                nc.vector.tensor_tensor(out=t_t[:, 0:8 * C], in0=t_t[:, 0:8 * C],
                                        in1=t_t[:, 8 * C:16 * C],
                                        op=mybir.AluOpType.add)
                nc.vector.tensor_tensor(out=t_t[:, 0:4 * C], in0=t_t[:, 0:4 * C],
                                        in1=t_t[:, 4 * C:8 * C],
                                        op=mybir.AluOpType.add)
                nc.vector.tensor_tensor(out=t_t[:, 0:2 * C], in0=t_t[:, 0:2 * C],
                                        in1=t_t[:, 2 * C:4 * C],
                                        op=mybir.AluOpType.add)
                r_t = r_pool.tile([P, C], mybir.dt.float32)
                nc.vector.tensor_tensor(out=r_t[:], in0=t_t[:, 0:C],
                                        in1=t_t[:, C:2 * C],
                                        op=mybir.AluOpType.add)

                nc.sync.dma_start(out=out_d.ap()[ch * P:(ch + 1) * P, :],
                                  in_=r_t[:])
    nc.compile()
    return nc


def _run(inputs, trace=False, trace_cores=None):
    """Returns (full_output [N,C,7,7] f32, exec_time_ns or None)."""
    from concourse.bass_utils import run_bass_kernel_spmd

    feats = (inputs["feat0"], inputs["feat1"], inputs["feat2"])
    boxes = inputs["boxes"]
    batch_ids = inputs["batch_ids"]
    feat_idx = inputs["feat_idx"]
    N = np.asarray(boxes).shape[0]

    table, _bases = _build_patch_table(feats)
    idx_all, w_all = _prep_indices_weights(boxes, batch_ids, feat_idx)

    act = (w_all != 0).any(-1)                      # [N,7,7]
    rois, phs, pws = np.nonzero(act)
    nact = len(rois)
    out_full = np.zeros((N, C, OUT, OUT), np.float32)
    if nact == 0:
        return out_full, None

    order = np.arange(nact)
    core_of = order % NCORES
    per_core = [np.nonzero(core_of == c)[0] for c in range(NCORES)]
    maxn = max(len(p) for p in per_core)
    nch = (maxn + P - 1) // P

    key = (table.shape[0], nch)
    if key not in _NEFF_CACHE:
        _NEFF_CACHE[key] = _build_program(table.shape[0], nch)
    nc = _NEFF_CACHE[key]

    in_maps = []
    for c in range(NCORES):
        sel = per_core[c]
        icore = np.zeros((nch * P, NSAMP), np.int32)
        wcore = np.zeros((nch * P, K), np.float32)
        icore[:len(sel)] = idx_all[rois[sel], phs[sel], pws[sel]]
        wcore[:len(sel)] = w_all[rois[sel], phs[sel], pws[sel]]
        in_maps.append({
            "table": table,
            "idx": icore.reshape(nch, P, NSAMP),
            "w": wcore.reshape(nch, P, K),
        })

    kwargs = {}
    if trace:
        kwargs["trace"] = True
        if trace_cores is not None:
            kwargs["trace_cores"] = trace_cores
    res = run_bass_kernel_spmd(nc, in_maps, core_ids=list(range(NCORES)),
                               **kwargs)

    for c in range(NCORES):
        sel = per_core[c]
        r = res.results[c]["out"][:len(sel)]
        out_full[rois[sel], :, phs[sel], pws[sel]] = r

    return out_full, getattr(res, "exec_time_ns", None)


def kernel(**inputs):
    out, _ = _run(inputs, trace=False)
    return out
